# revision 35
# baseline (speedup 1.0000x reference)
"""Trainium2 Bass kernel for a 3-layer GCN encoder over two graphs (x, y).

Dense-adjacency formulation:
  GCNConv(h) = D^-1/2 (A+I) D^-1/2 (h @ W) + b
  With Acnt the self-loop-augmented adjacency-count matrix and dinv = deg^-1/2:
      Hhat_1   = dinv * x                   (host, shipped fp8)
      P_l      = Acnt @ Hhat_l              (PE matmul, dominant cost)
      S_l      = dinv * P_l                 (dst-side norm)
      z_l      = S_l @ W_l + b_l            (PE matmul; bias via rank-1 matmul)
      Hhat_l+1 = dinv * relu(z_l)           (src-side norm of next layer)
  Output layer: out = S_3 @ W_3 + b_3.

Sharding (4+4 graph split): cores 0-3 own graph x, cores 4-7 graph y; each
core owns a 2560-row (2500 real) dst shard of its graph. Acnt is exact small
integer counts shipped fp8e4. Layers 1-2 consume H in fp8 and run the
aggregation with DoubleRow fp8 matmuls (2 k-tiles per instruction, ~2x);
layer 3 consumes H in bf16 (fp8 there would push rel-err past the gate).
Per-group 4-rank AllGathers replicate H between layers and hide under the
m-loop compute of the same layer.

Node ids are renumbered into a padded space of 10240 = 4*2560 so all tiles
are 128-multiples and the AllGather output is directly the packed SBUF
image of H.
"""

import numpy as np
import ml_dtypes

import concourse.bass as bass
import concourse.tile as tile
from concourse import bacc, mybir
import concourse.bass_utils as bass_utils
from concourse.masks import make_identity

BF16 = ml_dtypes.bfloat16
FP8 = ml_dtypes.float8_e4m3  # adjacency counts are <= 2: exact in e4m3

P = 128          # partitions / tile edge
NC = 8           # cores
NG = 4           # cores per graph group
N_NODES = 10000
SHARD = 2500     # real nodes per core
SHP = 2560       # padded nodes per core
NPAD = NG * SHP  # 10240
KT = NPAD // P   # 80 k-tiles over src nodes
MT = SHP // P    # 20 m-tiles per core
F = 256          # in/hidden feature width
FO = 128         # output feature width
ABUFS = 8        # A-slab prefetch depth

_NC_CACHE = {}


# ----------------------------------------------------------------------------
# Host-side graph preprocessing (index/static work only)
# ----------------------------------------------------------------------------

def _pad_ids(n):
    return (n // SHARD) * SHP + (n % SHARD)


def _prep_graph(x, edge_index, Ws, bs):
    """Returns per-core input maps (4 cores) for one graph."""
    src = edge_index[0].astype(np.int64)
    dst = edge_index[1].astype(np.int64)
    loop = np.arange(N_NODES, dtype=np.int64)
    src = np.concatenate([src, loop])
    dst = np.concatenate([dst, loop])
    sp = _pad_ids(src)
    dp = _pad_ids(dst)

    deg = np.zeros(NPAD, np.float32)
    np.add.at(deg, dp, np.float32(1.0))
    dinv = np.zeros(NPAD, np.float32)
    nz = deg > 0
    dinv[nz] = 1.0 / np.sqrt(deg[nz])

    at = np.zeros((NPAD, NPAD), np.float32)   # [src, dst] = A^T counts
    np.add.at(at, (sp, dp), np.float32(1.0))

    h1 = np.zeros((NPAD, F), np.float32)
    h1[_pad_ids(loop)] = x * dinv[_pad_ids(loop)][:, None]
    h1_img = np.ascontiguousarray(
        h1.reshape(KT, P, F).transpose(1, 0, 2)
    ).astype(FP8)                              # [P, KT, F]

    def w_img(W, fo):
        kf = W.shape[0] // P
        return np.ascontiguousarray(
            W.reshape(kf, P, fo).transpose(1, 0, 2).reshape(P, kf * fo)
        ).astype(BF16)

    w_imgs = [w_img(Ws[0], F), w_img(Ws[1], F), w_img(Ws[2], FO)]
    b_rows = [bs[0].reshape(1, F).astype(BF16),
              bs[1].reshape(1, F).astype(BF16),
              bs[2].reshape(1, FO).astype(BF16)]

    maps = []
    for c in range(NG):
        shard = at[:, c * SHP:(c + 1) * SHP]  # [NPAD src, SHP dst]
        slab = np.ascontiguousarray(
            shard.reshape(KT, P, MT, P).transpose(2, 1, 0, 3)
        ).astype(FP8)                          # [MT, P, KT, P]
        maps.append({
            "at": slab,
            "h1": h1_img,
            "w0": w_imgs[0], "w1": w_imgs[1], "w2": w_imgs[2],
            "b0": b_rows[0], "b1": b_rows[1], "b2": b_rows[2],
            "dinv": np.ascontiguousarray(
                dinv[c * SHP:(c + 1) * SHP].reshape(MT, P).T
            ).astype(np.float32),              # [P, MT]
        })
    return maps


def prep_in_maps(x, x_edge_index, y, y_edge_index,
                 W1x, b1x, W2x, b2x, W3x, b3x,
                 W1y, b1y, W2y, b2y, W3y, b3y):
    mx = _prep_graph(
        np.asarray(x, np.float32), np.asarray(x_edge_index),
        (np.asarray(W1x), np.asarray(W2x), np.asarray(W3x)),
        (np.asarray(b1x), np.asarray(b2x), np.asarray(b3x)))
    my = _prep_graph(
        np.asarray(y, np.float32), np.asarray(y_edge_index),
        (np.asarray(W1y), np.asarray(W2y), np.asarray(W3y)),
        (np.asarray(b1y), np.asarray(b2y), np.asarray(b3y)))
    return mx + my


def _unshard(z_imgs):
    """4 per-core [P, MT*FO] images -> [N_NODES, FO] for one graph."""
    rows = []
    for z in z_imgs:
        r = z.reshape(P, MT, FO).transpose(1, 0, 2).reshape(SHP, FO)
        rows.append(r[:SHARD])
    return np.concatenate(rows, axis=0)


# ----------------------------------------------------------------------------
# Device kernel (SPMD: same program on all 8 cores; data differs per core)
# ----------------------------------------------------------------------------

def _build_nc():
    if "nc" in _NC_CACHE:
        return _NC_CACHE["nc"]
    nc = bacc.Bacc("TRN2", target_bir_lowering=False, debug=False, num_devices=NC)
    dt = mybir.dt

    at = nc.dram_tensor("at", [MT, P, KT, P], dt.float8e4,
                        kind="ExternalInput").ap()
    h1 = nc.dram_tensor("h1", [P, KT, F], dt.float8e4,
                        kind="ExternalInput").ap()
    w_ap = [nc.dram_tensor(f"w{i}", [P, 2 * (FO if i == 2 else F)], dt.bfloat16,
                           kind="ExternalInput").ap() for i in range(3)]
    b_ap = [nc.dram_tensor(f"b{i}", [1, FO if i == 2 else F], dt.bfloat16,
                           kind="ExternalInput").ap() for i in range(3)]
    dinv = nc.dram_tensor("dinv", [P, MT], dt.float32, kind="ExternalInput").ap()
    zout = nc.dram_tensor("z", [P, MT * FO], dt.float32,
                          kind="ExternalOutput").ap()

    groups = [list(range(NG)), list(range(NG, NC))]

    with tile.TileContext(nc) as tc:
        with (
            tc.tile_pool(name="persist", bufs=1) as pers,
            tc.tile_pool(name="aslab", bufs=ABUFS) as apool,
            tc.tile_pool(name="work", bufs=4) as wk,
            tc.tile_pool(name="pagg", bufs=2, space="PSUM") as pagg,
            tc.tile_pool(name="ptr", bufs=2, space="PSUM") as ptr,
            tc.tile_pool(name="pg", bufs=2, space="PSUM") as pg,
            tc.tile_pool(name="dram", bufs=1, space="DRAM") as dp,
        ):
            H8 = pers.tile([P, KT, F], dt.float8e4)    # layers 1-2 H (fp8)
            H16 = pers.tile([P, KT, F], dt.bfloat16)   # layer 3 H (bf16)
            Hown8 = pers.tile([P, MT * F], dt.float8e4)
            Hown16 = pers.tile([P, MT * F], dt.bfloat16)
            Wt = [pers.tile([P, 2 * (FO if i == 2 else F)], dt.bfloat16,
                            name=f"wt{i}") for i in range(3)]
            Bt = [pers.tile([1, FO if i == 2 else F], dt.bfloat16,
                            name=f"bt{i}") for i in range(3)]
            Dv = pers.tile([P, MT], dt.float32)
            ident = pers.tile([P, P], dt.bfloat16)
            ones = pers.tile([1, P], dt.bfloat16)

            make_identity(nc, ident[:])
            nc.gpsimd.memset(ones[:], 1.0)

            # A-slab prefetch, software-pipelined ABUFS deep: slab i+ABUFS's
            # doorbell is emitted at iteration i so boundary stalls in the
            # scalar stream can't delay the slab the PE needs next. Pool WAR
            # deps pace the stream automatically.
            slab_tiles = []

            def prefetch_slab(queue=None):
                i = len(slab_tiles)
                if i >= 3 * MT:
                    return
                t = apool.tile([P, KT, P], dt.float8e4, tag="aslab",
                               name=f"aslab{i}")
                (queue or nc.scalar).dma_start(t[:], at[i % MT])
                slab_tiles.append(t)

            # Startup: PE needs h1 (fp8, 2.6MB) + slab0 + w0/b0/dinv early.
            for _ in range(3):
                prefetch_slab(nc.gpsimd)
            # Warmup collective: prepays comm-channel setup / entry-barrier
            # cost so the first real AllGather isn't delayed by it.
            wbin = dp.tile([1, P], dt.float8e4, tag="wbin")
            wbout = dp.tile([NG, P], dt.float8e4, tag="wbout")
            nc.sync.dma_start(wbin[:], h1[0:1, 0, :P])
            nc.gpsimd.collective_compute(
                "AllGather",
                mybir.AluOpType.bypass,
                replica_groups=groups,
                ins=[wbin[:].opt()],
                outs=[wbout[:].opt()],
            )
            KH = KT // 4
            for r in range(2):
                nc.sync.dma_start(H8[:, r * KH:(r + 1) * KH, :],
                                  h1[:, r * KH:(r + 1) * KH, :])
            for r in range(2, 4):
                nc.scalar.dma_start(H8[:, r * KH:(r + 1) * KH, :],
                                    h1[:, r * KH:(r + 1) * KH, :])
            nc.scalar.dma_start(Dv[:], dinv)
            for i in range(3):
                nc.scalar.dma_start(Wt[i][:], w_ap[i])
                nc.scalar.dma_start(Bt[i][:], b_ap[i])
            for _ in range(ABUFS - 3):
                prefetch_slab()

            # AG chunk boundaries (m-tile index, even so DR k-pairs never
            # straddle a chunk). Each layer's k-loop consumes k-tiles in the
            # chunk order its H image arrives in, so the m-loop can start
            # while late AG chunks are still in flight.
            BOUNDS = [0, 6, 10, 16, 20]

            def chunk_k_order(step):
                order = []
                for ci in range(len(BOUNDS) - 1):
                    for r in range(NG):
                        order.extend(
                            range((r * MT + BOUNDS[ci]) // step,
                                  (r * MT + BOUNDS[ci + 1]) // step))
                return order

            KORD_DR = chunk_k_order(2)   # 40 DoubleRow pair indices
            KORD_BF = chunk_k_order(1)   # 80 plain k indices

            for layer in range(3):
                fo = FO if layer == 2 else F
                Wl = Wt[layer]
                Bl = Bt[layer]
                for m in range(MT):
                    a_slab = slab_tiles[layer * MT + m]
                    pP = pagg.tile([P, F], dt.float32, tag="agg")
                    if layer < 2:
                        order = (range(KT // 2) if layer == 0 else KORD_DR)
                        for i, k2 in enumerate(order):
                            nc.tensor.matmul(
                                pP[:],
                                lhsT=a_slab[:, 2 * k2:2 * k2 + 2, :],
                                rhs=H8[:, 2 * k2:2 * k2 + 2, :],
                                start=(i == 0),
                                stop=(i == KT // 2 - 1),
                                perf_mode=mybir.MatmulPerfMode.DoubleRow,
                            )
                    else:
                        for i, k in enumerate(KORD_BF):
                            nc.tensor.matmul(
                                pP[:],
                                lhsT=a_slab[:, k, :],
                                rhs=H16[:, k, :],
                                start=(i == 0),
                                stop=(i == KT - 1),
                            )
                    prefetch_slab()
                    S = wk.tile([P, F], dt.bfloat16, tag="S")
                    nc.vector.tensor_scalar_mul(S[:], pP[:], Dv[:, m:m + 1])
                    gps = pg.tile([P, fo], dt.float32, tag="g")
                    for kf in range(2):
                        pT = ptr.tile([P, P], dt.bfloat16, tag="tr")
                        nc.tensor.transpose(
                            pT[:], S[:, kf * P:(kf + 1) * P], ident[:]
                        )
                        STk = wk.tile([P, P], dt.bfloat16, tag="ST")
                        nc.vector.tensor_copy(STk[:], pT[:])
                        nc.tensor.matmul(
                            gps[:],
                            lhsT=STk[:],
                            rhs=Wl[:, kf * fo:(kf + 1) * fo],
                            start=(kf == 0),
                            stop=False,
                        )
                    nc.tensor.matmul(
                        gps[:],
                        lhsT=ones[:1, :],
                        rhs=Bl[:1, :fo],
                        start=False,
                        stop=True,
                    )
                    if layer == 0:
                        nc.scalar.activation(
                            Hown8[:, m * F:(m + 1) * F],
                            gps[:],
                            mybir.ActivationFunctionType.Relu,
                            scale=Dv[:, m:m + 1],
                        )
                    elif layer == 1:
                        nc.scalar.activation(
                            Hown16[:, m * F:(m + 1) * F],
                            gps[:],
                            mybir.ActivationFunctionType.Relu,
                            scale=Dv[:, m:m + 1],
                        )
                    else:
                        zt = wk.tile([P, FO], dt.float32, tag="zt")
                        nc.vector.tensor_copy(zt[:], gps[:])
                        nc.sync.dma_start(
                            zout[:, m * FO:(m + 1) * FO], zt[:]
                        )
                if layer < 2:
                    # Chunked AllGather of this layer's H across the 4-rank
                    # group. Chunks fire as their Hown tiles complete (deps
                    # are per-chunk), so all but the last hide under the
                    # m-loop; chunks shrink toward the end to cut the exposed
                    # tail before the next layer can start. Collectives sit
                    # alone on the gpsimd stream (back-to-back issue);
                    # agin/reload DMAs ride the sync queue, each agin emitted
                    # before the previous chunk's reloads so a late collective
                    # can't delay the next chunk's input.
                    Hown = Hown8 if layer == 0 else Hown16
                    Hdst = H8 if layer == 0 else H16
                    hdt = dt.float8e4 if layer == 0 else dt.bfloat16
                    bounds = BOUNDS
                    nch = len(bounds) - 1
                    agouts = []
                    for ci in range(nch):
                        a, b = bounds[ci], bounds[ci + 1]
                        w = (b - a) * F
                        agin = dp.tile([P, w], hdt, tag=f"agin{layer}{ci}")
                        agout = dp.tile([NG * P, w], hdt,
                                        tag=f"agout{layer}{ci}")
                        nc.sync.dma_start(agin[:], Hown[:, a * F:b * F])
                        nc.gpsimd.collective_compute(
                            "AllGather",
                            mybir.AluOpType.bypass,
                            replica_groups=groups,
                            ins=[agin[:].opt()],
                            outs=[agout[:].opt()],
                        )
                        agouts.append(agout)
                        if ci > 0:
                            pa, pb = bounds[ci - 1], bounds[ci]
                            for r in range(NG):
                                nc.sync.dma_start(
                                    Hdst[:, r * MT + pa:r * MT + pb, :],
                                    agouts[ci - 1][r * P:(r + 1) * P, :],
                                )
                    pa, pb = bounds[nch - 1], bounds[nch]
                    for r in range(NG):
                        nc.sync.dma_start(
                            Hdst[:, r * MT + pa:r * MT + pb, :],
                            agouts[nch - 1][r * P:(r + 1) * P, :],
                        )
    nc.compile()
    _NC_CACHE["nc"] = nc
    return nc


# ----------------------------------------------------------------------------
# Entry point
# ----------------------------------------------------------------------------

def kernel(x, x_edge_index, y, y_edge_index,
           W1x, b1x, W2x, b2x, W3x, b3x,
           W1y, b1y, W2y, b2y, W3y, b3y,
           _trace=False, _trace_cores=None):
    in_maps = prep_in_maps(x, x_edge_index, y, y_edge_index,
                           W1x, b1x, W2x, b2x, W3x, b3x,
                           W1y, b1y, W2y, b2y, W3y, b3y)
    nc = _build_nc()
    kw = {}
    if _trace:
        kw = dict(trace=True, trace_cores=_trace_cores or [0])
    res = bass_utils.run_bass_kernel_spmd(
        nc, in_maps, core_ids=list(range(NC)), **kw
    )
    z = [res.results[c]["z"] for c in range(NC)]
    out_x = _unshard(z[:NG])
    out_y = _unshard(z[NG:])
    if _trace:
        kernel._last_result = res
    return out_x, out_y


# revision 39
# speedup vs baseline: 1.0156x; 1.0156x over previous
"""Trainium2 Bass kernel for a 3-layer GCN encoder over two graphs (x, y).

Dense-adjacency formulation:
  GCNConv(h) = D^-1/2 (A+I) D^-1/2 (h @ W) + b
  With Acnt the self-loop-augmented adjacency-count matrix and dinv = deg^-1/2:
      Hhat_1   = dinv * x                   (host, shipped fp8)
      P_l      = Acnt @ Hhat_l              (PE matmul, dominant cost)
      S_l      = dinv * P_l                 (dst-side norm)
      z_l      = S_l @ W_l + b_l            (PE matmul; bias via rank-1 matmul)
      Hhat_l+1 = dinv * relu(z_l)           (src-side norm of next layer)
  Output layer: out = S_3 @ W_3 + b_3.

Sharding (4+4 graph split): cores 0-3 own graph x, cores 4-7 graph y; each
core owns a 2560-row (2500 real) dst shard of its graph. Acnt is exact small
integer counts shipped fp8e4. Layers 1-2 consume H in fp8 and run the
aggregation with DoubleRow fp8 matmuls (2 k-tiles per instruction, ~2x);
layer 3 consumes H in bf16 (fp8 there would push rel-err past the gate).
Per-group 4-rank AllGathers replicate H between layers and hide under the
m-loop compute of the same layer.

Node ids are renumbered into a padded space of 10240 = 4*2560 so all tiles
are 128-multiples and the AllGather output is directly the packed SBUF
image of H.
"""

import numpy as np
import ml_dtypes

import concourse.bass as bass
import concourse.tile as tile
from concourse import bacc, mybir
import concourse.bass_utils as bass_utils
from concourse.masks import make_identity

BF16 = ml_dtypes.bfloat16
FP8 = ml_dtypes.float8_e4m3  # adjacency counts are <= 2: exact in e4m3

P = 128          # partitions / tile edge
NC = 8           # cores
NG = 4           # cores per graph group
N_NODES = 10000
SHARD = 2500     # real nodes per core
SHP = 2560       # padded nodes per core
NPAD = NG * SHP  # 10240
KT = NPAD // P   # 80 k-tiles over src nodes
MT = SHP // P    # 20 m-tiles per core
F = 256          # in/hidden feature width
FO = 128         # output feature width
ABUFS = 6        # A-slab streaming prefetch depth
KKEEP = 4        # slabs cached in SBUF across all 3 layers (same A per layer)

_NC_CACHE = {}


# ----------------------------------------------------------------------------
# Host-side graph preprocessing (index/static work only)
# ----------------------------------------------------------------------------

def _pad_ids(n):
    return (n // SHARD) * SHP + (n % SHARD)


def _prep_graph(x, edge_index, Ws, bs):
    """Returns per-core input maps (4 cores) for one graph."""
    src = edge_index[0].astype(np.int64)
    dst = edge_index[1].astype(np.int64)
    loop = np.arange(N_NODES, dtype=np.int64)
    src = np.concatenate([src, loop])
    dst = np.concatenate([dst, loop])
    sp = _pad_ids(src)
    dp = _pad_ids(dst)

    deg = np.zeros(NPAD, np.float32)
    np.add.at(deg, dp, np.float32(1.0))
    dinv = np.zeros(NPAD, np.float32)
    nz = deg > 0
    dinv[nz] = 1.0 / np.sqrt(deg[nz])

    at = np.zeros((NPAD, NPAD), np.float32)   # [src, dst] = A^T counts
    np.add.at(at, (sp, dp), np.float32(1.0))

    h1 = np.zeros((NPAD, F), np.float32)
    h1[_pad_ids(loop)] = x * dinv[_pad_ids(loop)][:, None]
    h1_img = np.ascontiguousarray(
        h1.reshape(KT, P, F).transpose(1, 0, 2)
    ).astype(FP8)                              # [P, KT, F]

    def w_img(W, fo):
        kf = W.shape[0] // P
        return np.ascontiguousarray(
            W.reshape(kf, P, fo).transpose(1, 0, 2).reshape(P, kf * fo)
        ).astype(BF16)

    w_imgs = [w_img(Ws[0], F), w_img(Ws[1], F), w_img(Ws[2], FO)]
    b_rows = [bs[0].reshape(1, F).astype(BF16),
              bs[1].reshape(1, F).astype(BF16),
              bs[2].reshape(1, FO).astype(BF16)]

    maps = []
    for c in range(NG):
        shard = at[:, c * SHP:(c + 1) * SHP]  # [NPAD src, SHP dst]
        slab = np.ascontiguousarray(
            shard.reshape(KT, P, MT, P).transpose(2, 1, 0, 3)
        ).astype(FP8)                          # [MT, P, KT, P]
        maps.append({
            "at": slab,
            "h1": h1_img,
            "w0": w_imgs[0], "w1": w_imgs[1], "w2": w_imgs[2],
            "b0": b_rows[0], "b1": b_rows[1], "b2": b_rows[2],
            "dinv": np.ascontiguousarray(
                dinv[c * SHP:(c + 1) * SHP].reshape(MT, P).T
            ).astype(np.float32),              # [P, MT]
        })
    return maps


def prep_in_maps(x, x_edge_index, y, y_edge_index,
                 W1x, b1x, W2x, b2x, W3x, b3x,
                 W1y, b1y, W2y, b2y, W3y, b3y):
    mx = _prep_graph(
        np.asarray(x, np.float32), np.asarray(x_edge_index),
        (np.asarray(W1x), np.asarray(W2x), np.asarray(W3x)),
        (np.asarray(b1x), np.asarray(b2x), np.asarray(b3x)))
    my = _prep_graph(
        np.asarray(y, np.float32), np.asarray(y_edge_index),
        (np.asarray(W1y), np.asarray(W2y), np.asarray(W3y)),
        (np.asarray(b1y), np.asarray(b2y), np.asarray(b3y)))
    return mx + my


def _unshard(z_imgs):
    """4 per-core [P, MT*FO] images -> [N_NODES, FO] for one graph."""
    rows = []
    for z in z_imgs:
        r = z.reshape(P, MT, FO).transpose(1, 0, 2).reshape(SHP, FO)
        rows.append(r[:SHARD])
    return np.concatenate(rows, axis=0)


# ----------------------------------------------------------------------------
# Device kernel (SPMD: same program on all 8 cores; data differs per core)
# ----------------------------------------------------------------------------

def _build_nc():
    if "nc" in _NC_CACHE:
        return _NC_CACHE["nc"]
    nc = bacc.Bacc("TRN2", target_bir_lowering=False, debug=False, num_devices=NC)
    dt = mybir.dt

    at = nc.dram_tensor("at", [MT, P, KT, P], dt.float8e4,
                        kind="ExternalInput").ap()
    h1 = nc.dram_tensor("h1", [P, KT, F], dt.float8e4,
                        kind="ExternalInput").ap()
    w_ap = [nc.dram_tensor(f"w{i}", [P, 2 * (FO if i == 2 else F)], dt.bfloat16,
                           kind="ExternalInput").ap() for i in range(3)]
    b_ap = [nc.dram_tensor(f"b{i}", [1, FO if i == 2 else F], dt.bfloat16,
                           kind="ExternalInput").ap() for i in range(3)]
    dinv = nc.dram_tensor("dinv", [P, MT], dt.float32, kind="ExternalInput").ap()
    zout = nc.dram_tensor("z", [P, MT * FO], dt.float32,
                          kind="ExternalOutput").ap()

    groups = [list(range(NG)), list(range(NG, NC))]

    with tile.TileContext(nc) as tc:
        with (
            tc.tile_pool(name="persist", bufs=1) as pers,
            tc.tile_pool(name="aslab", bufs=ABUFS) as apool,
            tc.tile_pool(name="work", bufs=4) as wk,
            tc.tile_pool(name="pagg", bufs=2, space="PSUM") as pagg,
            tc.tile_pool(name="ptr", bufs=2, space="PSUM") as ptr,
            tc.tile_pool(name="pg", bufs=2, space="PSUM") as pg,
            tc.tile_pool(name="dram", bufs=1, space="DRAM") as dp,
        ):
            H8 = pers.tile([P, KT, F], dt.float8e4)    # layers 1-2 H (fp8)
            H16 = pers.tile([P, KT, F], dt.bfloat16)   # layer 3 H (bf16)
            Hown8 = pers.tile([P, MT * F], dt.float8e4)
            Hown16 = pers.tile([P, MT * F], dt.bfloat16)
            Wt = [pers.tile([P, 2 * (FO if i == 2 else F)], dt.bfloat16,
                            name=f"wt{i}") for i in range(3)]
            Bt = [pers.tile([1, FO if i == 2 else F], dt.bfloat16,
                            name=f"bt{i}") for i in range(3)]
            Dv = pers.tile([P, MT], dt.float32)
            ident = pers.tile([P, P], dt.bfloat16)
            ones = pers.tile([1, P], dt.bfloat16)
            # Cached A slabs (m < KKEEP), loaded once and reused by all 3
            # layers — A is layer-invariant, so this trims slab DMA out of
            # the HBM-saturated DR-layer windows. Allocated at the end of
            # pers: inserting tiles earlier shifts every SBUF offset and was
            # measured to slow all matmuls by ~18%.
            Akeep = [pers.tile([P, KT, P], dt.float8e4, name=f"akeep{k}")
                     for k in range(KKEEP)]

            make_identity(nc, ident[:])
            nc.gpsimd.memset(ones[:], 1.0)

            # A-slab streams, software-pipelined ABUFS deep: a slab doorbell
            # is emitted every m-iteration, so boundary stalls in the scalar
            # stream can't delay the slab the PE needs next. Pool WAR deps
            # pace the stream automatically. m < KKEEP hits the cache.
            slab_tiles = [None] * (3 * MT)
            for i in range(3 * MT):
                if i % MT < KKEEP:
                    slab_tiles[i] = Akeep[i % MT]
            stream_ms = [i for i in range(3 * MT) if i % MT >= KKEEP]
            stream_pos = [0]

            def prefetch_slab(queue=None):
                if stream_pos[0] >= len(stream_ms):
                    return
                i = stream_ms[stream_pos[0]]
                stream_pos[0] += 1
                t = apool.tile([P, KT, P], dt.float8e4, tag="aslab",
                               name=f"aslab{i}")
                (queue or nc.scalar).dma_start(t[:], at[i % MT])
                slab_tiles[i] = t

            # Startup: PE needs h1 (fp8, 2.6MB) + slab0 + w0/b0/dinv early.
            for k in range(2):
                nc.gpsimd.dma_start(Akeep[k][:], at[k])
            # Warmup collective: prepays comm-channel setup / entry-barrier
            # cost so the first real AllGather isn't delayed by it.
            wbin = dp.tile([1, P], dt.float8e4, tag="wbin")
            wbout = dp.tile([NG, P], dt.float8e4, tag="wbout")
            nc.sync.dma_start(wbin[:], h1[0:1, 0, :P])
            nc.gpsimd.collective_compute(
                "AllGather",
                mybir.AluOpType.bypass,
                replica_groups=groups,
                ins=[wbin[:].opt()],
                outs=[wbout[:].opt()],
            )
            KH = KT // 4
            for r in range(2):
                nc.sync.dma_start(H8[:, r * KH:(r + 1) * KH, :],
                                  h1[:, r * KH:(r + 1) * KH, :])
            for r in range(2, 4):
                nc.scalar.dma_start(H8[:, r * KH:(r + 1) * KH, :],
                                    h1[:, r * KH:(r + 1) * KH, :])
            nc.scalar.dma_start(Dv[:], dinv)
            for i in range(3):
                nc.scalar.dma_start(Wt[i][:], w_ap[i])
                nc.scalar.dma_start(Bt[i][:], b_ap[i])
            for k in range(2, KKEEP):
                nc.scalar.dma_start(Akeep[k][:], at[k])
            prefetch_slab(nc.gpsimd)
            for _ in range(2):
                prefetch_slab()

            # AG chunk boundaries (m-tile index, even so DR k-pairs never
            # straddle a chunk). Each layer's k-loop consumes k-tiles in the
            # chunk order its H image arrives in, so the m-loop can start
            # while late AG chunks are still in flight.
            BOUNDS = [0, 6, 10, 16, 20]

            def chunk_k_order(step):
                order = []
                for ci in range(len(BOUNDS) - 1):
                    for r in range(NG):
                        order.extend(
                            range((r * MT + BOUNDS[ci]) // step,
                                  (r * MT + BOUNDS[ci + 1]) // step))
                return order

            KORD_DR = chunk_k_order(2)   # 40 DoubleRow pair indices
            KORD_BF = chunk_k_order(1)   # 80 plain k indices

            for layer in range(3):
                fo = FO if layer == 2 else F
                Wl = Wt[layer]
                Bl = Bt[layer]
                for m in range(MT):
                    a_slab = slab_tiles[layer * MT + m]
                    pP = pagg.tile([P, F], dt.float32, tag="agg")
                    if layer < 2:
                        order = (range(KT // 2) if layer == 0 else KORD_DR)
                        for i, k2 in enumerate(order):
                            nc.tensor.matmul(
                                pP[:],
                                lhsT=a_slab[:, 2 * k2:2 * k2 + 2, :],
                                rhs=H8[:, 2 * k2:2 * k2 + 2, :],
                                start=(i == 0),
                                stop=(i == KT // 2 - 1),
                                perf_mode=mybir.MatmulPerfMode.DoubleRow,
                            )
                    else:
                        for i, k in enumerate(KORD_BF):
                            nc.tensor.matmul(
                                pP[:],
                                lhsT=a_slab[:, k, :],
                                rhs=H16[:, k, :],
                                start=(i == 0),
                                stop=(i == KT - 1),
                            )
                    prefetch_slab()
                    S = wk.tile([P, F], dt.bfloat16, tag="S")
                    nc.vector.tensor_scalar_mul(S[:], pP[:], Dv[:, m:m + 1])
                    gps = pg.tile([P, fo], dt.float32, tag="g")
                    for kf in range(2):
                        pT = ptr.tile([P, P], dt.bfloat16, tag="tr")
                        nc.tensor.transpose(
                            pT[:], S[:, kf * P:(kf + 1) * P], ident[:]
                        )
                        STk = wk.tile([P, P], dt.bfloat16, tag="ST")
                        nc.vector.tensor_copy(STk[:], pT[:])
                        nc.tensor.matmul(
                            gps[:],
                            lhsT=STk[:],
                            rhs=Wl[:, kf * fo:(kf + 1) * fo],
                            start=(kf == 0),
                            stop=False,
                        )
                    nc.tensor.matmul(
                        gps[:],
                        lhsT=ones[:1, :],
                        rhs=Bl[:1, :fo],
                        start=False,
                        stop=True,
                    )
                    if layer == 0:
                        nc.scalar.activation(
                            Hown8[:, m * F:(m + 1) * F],
                            gps[:],
                            mybir.ActivationFunctionType.Relu,
                            scale=Dv[:, m:m + 1],
                        )
                    elif layer == 1:
                        nc.scalar.activation(
                            Hown16[:, m * F:(m + 1) * F],
                            gps[:],
                            mybir.ActivationFunctionType.Relu,
                            scale=Dv[:, m:m + 1],
                        )
                    else:
                        zt = wk.tile([P, FO], dt.float32, tag="zt")
                        nc.vector.tensor_copy(zt[:], gps[:])
                        nc.sync.dma_start(
                            zout[:, m * FO:(m + 1) * FO], zt[:]
                        )
                if layer < 2:
                    # Chunked AllGather of this layer's H across the 4-rank
                    # group. Chunks fire as their Hown tiles complete (deps
                    # are per-chunk), so all but the last hide under the
                    # m-loop; chunks shrink toward the end to cut the exposed
                    # tail before the next layer can start. Collectives sit
                    # alone on the gpsimd stream (back-to-back issue);
                    # agin/reload DMAs ride the sync queue, each agin emitted
                    # before the previous chunk's reloads so a late collective
                    # can't delay the next chunk's input.
                    Hown = Hown8 if layer == 0 else Hown16
                    Hdst = H8 if layer == 0 else H16
                    hdt = dt.float8e4 if layer == 0 else dt.bfloat16
                    bounds = BOUNDS
                    nch = len(bounds) - 1
                    agouts = []
                    for ci in range(nch):
                        a, b = bounds[ci], bounds[ci + 1]
                        w = (b - a) * F
                        agin = dp.tile([P, w], hdt, tag=f"agin{layer}{ci}")
                        agout = dp.tile([NG * P, w], hdt,
                                        tag=f"agout{layer}{ci}")
                        nc.sync.dma_start(agin[:], Hown[:, a * F:b * F])
                        nc.gpsimd.collective_compute(
                            "AllGather",
                            mybir.AluOpType.bypass,
                            replica_groups=groups,
                            ins=[agin[:].opt()],
                            outs=[agout[:].opt()],
                        )
                        agouts.append(agout)
                        if ci > 0:
                            pa, pb = bounds[ci - 1], bounds[ci]
                            for r in range(NG):
                                nc.sync.dma_start(
                                    Hdst[:, r * MT + pa:r * MT + pb, :],
                                    agouts[ci - 1][r * P:(r + 1) * P, :],
                                )
                    pa, pb = bounds[nch - 1], bounds[nch]
                    for r in range(NG):
                        nc.sync.dma_start(
                            Hdst[:, r * MT + pa:r * MT + pb, :],
                            agouts[nch - 1][r * P:(r + 1) * P, :],
                        )
    nc.compile()
    _NC_CACHE["nc"] = nc
    return nc


# ----------------------------------------------------------------------------
# Entry point
# ----------------------------------------------------------------------------

def kernel(x, x_edge_index, y, y_edge_index,
           W1x, b1x, W2x, b2x, W3x, b3x,
           W1y, b1y, W2y, b2y, W3y, b3y,
           _trace=False, _trace_cores=None):
    in_maps = prep_in_maps(x, x_edge_index, y, y_edge_index,
                           W1x, b1x, W2x, b2x, W3x, b3x,
                           W1y, b1y, W2y, b2y, W3y, b3y)
    nc = _build_nc()
    kw = {}
    if _trace:
        kw = dict(trace=True, trace_cores=_trace_cores or [0])
    res = bass_utils.run_bass_kernel_spmd(
        nc, in_maps, core_ids=list(range(NC)), **kw
    )
    z = [res.results[c]["z"] for c in range(NC)]
    out_x = _unshard(z[:NG])
    out_y = _unshard(z[NG:])
    if _trace:
        kernel._last_result = res
    return out_x, out_y
